# revision 7
# baseline (speedup 1.0000x reference)
"""Multi-head attention (B=2, T=2048, C=1024, H=16) on 8 TRN2 NeuronCores.

Sharding: core c = (b, g) with b = c // 4 (data parallel over batch),
g = c % 4 (tensor parallel over head groups of 4 heads = 256 cols).
Wq/Wk/Wv are column-sharded, Wp row-sharded (Megatron); the host sums the
4 partial output projections per batch and adds the bias.

v2 layout/schedule (all hardcoded for the fixed problem shape):
  - host passes x^T [C, T] so projections need no on-device transpose
  - phase A runs as three sub-phases (Q, K, V), each double-buffered in
    PSUM, so weights stay hot and the DMA prefetch hides under compute
  - QT/KT produced as [cols, T] bf16 (partition = head-dim), V as
    [T, cols] packed into vaug with a trailing ones column per head so
    row 64 of O^T is the softmax denominator
  - scores are built transposed, S^T[k, q] = K_h^T.T @ Q_h^T, one
    128-row k-chunk at a time; exp runs on ACT (no max subtraction --
    with these input scales |S| <= ~2), mask is a bf16 {0,1} multiply
  - normalization: reciprocal_approx_fast on the denominator row,
    broadcast to 64 partitions with a K=1 matmul, then a single DVE
    multiply that evacuates PSUM into bf16; the whole normalization of
    round r is emitted inside round r+1's k-loop so no engine stalls at
    round boundaries
  - O^T for odd heads is staged in SBUF and DMA-copied to partitions
    64..127, giving a [128, T] head-pair layout; the output projection
    then contracts 128 rows per step (2 steps for 256 local cols) and is
    interleaved with the following q-half's attention rounds
"""
import numpy as np
import ml_dtypes

import bass_rust
import concourse.bass as bass
import concourse.mybir as mybir
import concourse.tile as tile
from concourse.bass_utils import run_bass_kernel_spmd
from concourse.vector_clock import ScopedClock

# ---------------------------------------------------------------------------
# Workaround: walrus rejects >~4 sync waits on one instruction; the Tile exit
# drain aggregates one wait per DMA queue/engine.  Spread them over a chain of
# single-wait NOPs on the sync engine before draining.
# ---------------------------------------------------------------------------


def _patched_drain_and_barrier(self, tick_clock, wait_clock):
    nc = self.nc
    probe = nc.sync.nop(nofuse=True)
    wait_clock.add_sem_waits(probe.ins, ScopedClock({None: tick_clock.global_clock}))
    waits = list(probe.ins.sync_info.on_wait) if probe.ins.sync_info else []
    probe.ins.sync_info = bass_rust.SyncInfo(
        on_wait=waits[:1], on_update=[]
    )
    for w in waits[1:]:
        n = nc.sync.nop(nofuse=True)
        n.ins.sync_info = bass_rust.SyncInfo(on_wait=[w], on_update=[])

    nc.sync.drain()
    nc.all_engine_barrier()
    assert self.sems is not None
    popped = nc._tile_sem_poison_stack.pop()
    assert popped is self._sem_poison
    nc.clear_and_free_semaphores(list(self.sems.allocated().values()))
    nc.all_engine_barrier()


tile.TileContext._drain_and_barrier = _patched_drain_and_barrier

_MAX_WAITS = 1


def _split_excess_waits(nc, limit=_MAX_WAITS):
    """Walrus codegen allows only ONE sync wait on compute instructions
    (more on CTRL, but be uniform).  For any instruction carrying more,
    peel the excess onto same-engine single-wait NOPs inserted immediately
    before it in the basic block."""
    n_new = 0
    for f in nc.m.functions:
        for bb in f.blocks:
            insts = bb.instructions
            out = []
            for inst in insts:
                si = inst.sync_info
                waits = list(si.on_wait) if si and si.on_wait else []
                if len(waits) > limit:
                    extra, keep = waits[:-limit], waits[-limit:]
                    inst.sync_info = bass_rust.SyncInfo(
                        on_wait=keep, on_update=list(si.on_update)
                    )
                    for j in range(0, len(extra), limit):
                        nop = mybir.InstNoOp(
                            name=f"waitsplit-{n_new}",
                            engine=inst.engine,
                            ins=[],
                            outs=[],
                            sync_info=bass_rust.SyncInfo(
                                on_wait=extra[j:j + limit], on_update=[]
                            ),
                        )
                        n_new += 1
                        out.append(nop)
                out.append(inst)
            if n_new:
                bb.instructions = out
    return n_new

# ---------------------------------------------------------------------------

B, T, C, H = 2, 2048, 1024, 16
GROUPS = 4                 # head groups (tensor parallel width per batch)
HG = H // GROUPS           # 4 heads per group
DH = C // H                # 64
COLS = HG * DH             # 256 local columns
KC = T // 128              # 16 k-chunks of 128
CC = C // 128              # 8 contraction chunks for the projections
QCB = T // 512             # 4 q chunks of 512

F32 = mybir.dt.float32
F32R = mybir.dt.float32r
BF16 = mybir.dt.bfloat16
FP8 = mybir.dt.float8e4


def _mm(nc, out, lhsT, rhs, start, stop):
    nc.tensor.matmul(out, lhsT, rhs, start=start, stop=stop)


def build_program(split_waits=True):
    nc = bass.Bass("TRN2", target_bir_lowering=False, debug=False, num_devices=8)

    xqT = nc.declare_dram_parameter("xqT", [C, T], FP8, isOutput=False)
    xkT = nc.declare_dram_parameter("xkT", [C, T], FP8, isOutput=False)
    xvT = nc.declare_dram_parameter("xvT", [C, T], BF16, isOutput=False)
    maskT = nc.declare_dram_parameter("maskT", [T, T], BF16, isOutput=False)
    wq = nc.declare_dram_parameter("wq", [C, COLS], FP8, isOutput=False)
    wk = nc.declare_dram_parameter("wk", [C, COLS], FP8, isOutput=False)
    wv = nc.declare_dram_parameter("wv", [C, COLS], BF16, isOutput=False)
    wp = nc.declare_dram_parameter("wp", [COLS, C], BF16, isOutput=False)
    ones_in = nc.declare_dram_parameter("ones", [1, DH], F32R, isOutput=False)
    y = nc.declare_dram_parameter("y", [T, C], F32, isOutput=True)

    with tile.TileContext(nc) as tc:
        import contextlib
        with contextlib.ExitStack() as ctx:
            persist = ctx.enter_context(tc.tile_pool(name="persist", bufs=1))

            # persistent SBUF tensors
            mask_sb = persist.tile([128, KC, T], BF16)        # 64 KB/part
            qt_sb = persist.tile([128, 2, T], BF16)           # 8 KB/part
            kt_sb = persist.tile([128, 2, T], BF16)           # 8 KB/part
            # partition-half-swapped copies: S matmuls alternate PE array
            # row halves per k-chunk so LDWEIGHTS overlaps the previous mm
            qt2_sb = persist.tile([128, 2, T], BF16)
            kt2_sb = persist.tile([128, 2, T], BF16)
            vaug_sb = persist.tile([128, KC, HG, DH + 1], BF16)
            ot_pair = persist.tile([128, 2, T], BF16)         # head-pair O^T
            wp_sb = persist.tile([128, 2, C], BF16)
            ones_sb = persist.tile([1, DH], F32R)
            warm_sb = persist.tile([128, 512], BF16)

            nc.sync.dma_start(ones_sb, ones_in[:, :])
            nc.vector.memset(vaug_sb[:, :, :, DH:DH + 1], 1.0)

            # ---------------- Phase A: projections (Q, K, V) ----------------
            with tc.tile_pool(name="wsb", bufs=1) as pw, \
                 tc.tile_pool(name="xchunks", bufs=3) as px:
                wq_sb = pw.tile([128, CC // 2, 2, COLS], FP8)
                wk_sb = pw.tile([128, CC // 2, 2, COLS], FP8)
                wv_sb = pw.tile([128, CC, COLS], BF16)

                # --- Q and K sub-phases, interleaved per q-chunk so the x
                # DMA stream and the matmuls stay in lockstep.  A single
                # ordered DMA queue (sync engine / HWDGE) keeps device order
                # deterministic; slot waits pace it naturally. ---
                with tc.tile_pool(name="psum_qk", bufs=3, space="PSUM") as ppqk, \
                     tc.tile_pool(name="psum_wm", bufs=1, space="PSUM") as ppwm:
                    nc.sync.dma_start(
                        wq_sb, wq.rearrange("(cc two p) n -> p cc two n",
                                            p=128, two=2))
                    nc.sync.dma_start(
                        wk_sb, wk.rearrange("(cc two p) n -> p cc two n",
                                            p=128, two=2))
                    nc.sync.dma_start(
                        wv_sb, wv.rearrange("(cc p) n -> p cc n", p=128))
                    # warm the PE p-state while the first x chunk loads
                    nc.vector.memset(warm_sb, 0.0)
                    warm_ps = ppwm.tile([128, 512], F32, tag="warm")
                    for _ in range(22):
                        _mm(nc, warm_ps, warm_sb[:, 0:128], warm_sb, True, True)
                    for qc in range(QCB):
                        qs = slice(qc * 512, (qc + 1) * 512)
                        for which, w_sb, x_dram, out_sb in (
                            ("q", wq_sb, xqT, qt_sb),
                            ("k", wk_sb, xkT, kt_sb),
                        ):
                            x_t = px.tile([128, CC // 2, 2, 512], FP8,
                                           tag=f"x{which}")
                            nc.sync.dma_start(
                                x_t,
                                x_dram[:, qs].rearrange(
                                    "(cc two p) q -> p cc two q", p=128, two=2))
                            o_ps = ppqk.tile([128, 2, 512], F32, tag="qk")
                            for cc in range(CC // 2):
                                st, sp = cc == 0, cc == CC // 2 - 1
                                for mh in range(2):
                                    m = slice(mh * 128, (mh + 1) * 128)
                                    nc.tensor.matmul(
                                        o_ps[:, mh], w_sb[:, cc, :, m],
                                        x_t[:, cc], start=st, stop=sp,
                                        perf_mode=mybir.MatmulPerfMode.DoubleRow)
                            # evacuate with the 1/256 fp8-range compensation
                            nc.vector.tensor_scalar_mul(
                                out_sb[:, 0, qs], o_ps[:, 0], 1.0 / 256.0)
                            nc.scalar.mul(out_sb[:, 1, qs], o_ps[:, 1], 1.0 / 256.0)

                # --- V sub-phase ---
                with tc.tile_pool(name="psum_v", bufs=2, space="PSUM") as ppv:
                    def mask_chunk(mc):
                        nc.sync.dma_start(
                            mask_sb[:, 2 * mc:2 * mc + 2],
                            maskT[mc * 256:(mc + 1) * 256, :].rearrange(
                                "(c p) q -> p c q", p=128))

                    for qc in range(QCB):
                        qs = slice(qc * 512, (qc + 1) * 512)
                        xv_t = px.tile([128, CC, 512], BF16, tag="xv")
                        nc.sync.dma_start(
                            xv_t, xvT[:, qs].rearrange("(cc p) q -> p cc q", p=128))
                        # stream mask chunks interleaved behind the xv slices
                        # (same Pool FIFO keeps device-order deterministic)
                        if qc == 1:
                            mask_chunk(0)
                        elif qc == 2:
                            mask_chunk(1)
                        elif qc == QCB - 1:
                            nc.sync.dma_start(
                                wp_sb, wp.rearrange("(p2 p) n -> p p2 n", p=128))
                            mask_chunk(2)
                            mask_chunk(3)
                            for a, b2 in ((qt2_sb, qt_sb), (kt2_sb, kt_sb)):
                                nc.sync.dma_start(a[0:64], b2[64:128])
                                nc.sync.dma_start(a[64:128], b2[0:64])
                            for mc in range(4, 8):
                                mask_chunk(mc)
                        v_ps = ppv.tile([128, 4, 512], F32, tag="v")
                        for cc in range(CC):
                            st, sp = cc == 0, cc == CC - 1
                            for tt in range(4):
                                _mm(nc, v_ps[:, tt, 0:COLS],
                                    xv_t[:, cc, tt * 128:(tt + 1) * 128],
                                    wv_sb[:, cc], st, sp)
                        for tt in range(4):
                            src = v_ps[:, tt, 0:COLS].rearrange(
                                "p (h d) -> p h d", h=HG)
                            dst = vaug_sb[:, qc * 4 + tt, :, 0:DH]
                            if tt % 2 == 0:
                                nc.scalar.copy(dst, src)
                            else:
                                nc.vector.tensor_copy(dst, src)

            # ---------------- Phase B + C interleaved ----------------
            with tc.tile_pool(name="pt", bufs=4) as ppt, \
                 tc.tile_pool(name="recip", bufs=2) as prc, \
                 tc.tile_pool(name="bcast", bufs=2) as pbc, \
                 tc.tile_pool(name="stage", bufs=2) as pst, \
                 tc.tile_pool(name="ysb", bufs=3) as py, \
                 tc.tile_pool(name="psum_s", bufs=2, space="PSUM") as pps, \
                 tc.tile_pool(name="psum_o", bufs=2, space="PSUM") as ppo:

                def norm_stages(task):
                    """Normalize + evacuate one finished round (h, qh).

                    Returns a list of emit-callbacks; the caller spreads them
                    across k-chunk positions of the NEXT round so no engine
                    FIFO ever blocks on the cross-engine chain.  The
                    denominator row is reshaped to 64 partitions via a tiny
                    SBUF-to-SBUF DMA so the (iterative, per-element) DVE
                    reciprocal touches only 16 elements per lane; the
                    reciprocals go back through a second tiny DMA and are
                    broadcast to 64 partitions with a K=1 matmul.
                    """
                    h, qh, ot_ps = task
                    pair, odd = h // 2, h % 2
                    qsl = slice(qh * 1024, (qh + 1) * 1024)
                    if odd:
                        dst = pst.tile([64, 1024], BF16, tag="stage")
                    else:
                        dst = ot_pair[0:64, pair, qsl]
                    st = {}

                    def s_dncopy():
                        dn = prc.tile([1, 1024], F32, tag="dn")
                        for j in range(2):
                            jj = slice(j * 512, (j + 1) * 512)
                            nc.vector.tensor_copy(dn[:, jj], ot_ps[DH:DH + 1, jj])
                        st["dn"] = dn

                    def s_reshape():
                        dn64 = prc.tile([64, 16], F32, tag="dn64")
                        nc.sync.dma_start(dn64, st["dn"])
                        st["dn64"] = dn64

                    def s_recip():
                        rc64 = prc.tile([64, 16], F32R, tag="rc64")
                        with nc.allow_low_precision(reason="softmax denom recip"):
                            nc.vector.reciprocal(rc64, st["dn64"])
                        st["rc64"] = rc64

                    def s_flatten():
                        rcf = prc.tile([1, 1024], F32R, tag="rcf")
                        nc.sync.dma_start(rcf, st["rc64"])
                        st["rcf"] = rcf

                    def s_bcmm():
                        for j in range(2):
                            jj = slice(j * 512, (j + 1) * 512)
                            bc_ps = pps.tile([DH, 512], F32, tag="s")
                            _mm(nc, bc_ps, ones_sb, st["rcf"][:, jj], True, True)
                            st[f"bcp{j}"] = bc_ps

                    def s_bccopy():
                        for j in range(2):
                            bc_sb = pbc.tile([DH, 512], F32, tag="bc")
                            nc.vector.tensor_copy(bc_sb, st[f"bcp{j}"])
                            st[f"bcs{j}"] = bc_sb

                    def s_mul(j):
                        jj = slice(j * 512, (j + 1) * 512)
                        nc.vector.tensor_mul(
                            dst[:, jj], ot_ps[0:DH, jj], st[f"bcs{j}"])

                    def s_dma():
                        if odd:
                            nc.sync.dma_start(ot_pair[64:128, pair, qsl], dst)

                    return [s_dncopy, s_reshape, s_recip, s_flatten, s_bcmm,
                            s_bccopy, lambda: s_mul(0), lambda: s_mul(1), s_dma]

                def emit_proj_tile(tt, tail=False):
                    """Output projection for one 128-token tile.  During the
                    attention rounds both evacuations go to DVE (ACT is the
                    bottleneck there); in the tail ACT is idle, so split."""
                    trange = slice(tt * 128, (tt + 1) * 128)
                    y_t = py.tile([128, C], F32, tag="y")
                    for nk in range(2):
                        ns = slice(nk * 512, (nk + 1) * 512)
                        y_ps = pps.tile([128, 512], F32, tag="s")
                        _mm(nc, y_ps, ot_pair[:, 0, trange], wp_sb[:, 0, ns],
                            True, False)
                        _mm(nc, y_ps, ot_pair[:, 1, trange], wp_sb[:, 1, ns],
                            False, True)
                        if nk == 0:
                            nc.scalar.copy(y_t[:, ns], y_ps)
                        else:
                            nc.vector.tensor_copy(y_t[:, ns], y_ps)
                    nc.sync.dma_start(y[trange, :], y_t)

                rounds = [(h, qh) for qh in range(2) for h in (1, 3, 0, 2)]
                NR = len(rounds)

                def score_chunk(ridx, kc):
                    """S^T matmuls + exp + mask for one (round, k-chunk).
                    Odd k-chunks (after round 0) use the partition-swapped
                    qt2/kt2 copies so consecutive S LDWEIGHTS hit different
                    PE row groups and overlap the in-flight matmul."""
                    h, qh = rounds[ridx]
                    swap = ridx >= 1 and kc % 2 == 1
                    pbase = ((h % 2) ^ (1 if swap else 0)) * 64
                    mh = h // 2
                    ktt = kt2_sb if swap else kt_sb
                    qtt = qt2_sb if swap else qt_sb
                    kt_h = ktt[pbase:pbase + 64, mh]
                    qt_h = qtt[pbase:pbase + 64, mh]
                    pt_t = ppt.tile([128, 1024], BF16, tag="pt")
                    ks = slice(kc * 128, (kc + 1) * 128)
                    s_ps = pps.tile([128, 1024], F32, tag="s")
                    for j in range(2):
                        qq = slice(qh * 1024 + j * 512,
                                   qh * 1024 + (j + 1) * 512)
                        _mm(nc, s_ps[:, j * 512:(j + 1) * 512],
                            kt_h[:, ks], qt_h[:, qq], True, True)
                    nc.scalar.activation(
                        pt_t, s_ps, mybir.ActivationFunctionType.Exp)
                    nc.vector.tensor_mul(
                        pt_t, pt_t, mask_sb[:, kc, qh * 1024:(qh + 1) * 1024])
                    return pt_t

                def warm_mms(n):
                    """Dummy matmuls that keep the PE HAM window busy when
                    real work is stalled on DMA-paced dependencies."""
                    w_ps = pps.tile([128, 512], F32, tag="s")
                    for _ in range(n):
                        _mm(nc, w_ps, warm_sb[:, 0:128], warm_sb, True, True)

                LOOK = 3
                pt_map = {}
                next_chunk = 0

                def ensure_chunks(upto):
                    nonlocal next_chunk
                    while next_chunk <= min(upto, NR * KC - 1):
                        pt_map[next_chunk] = score_chunk(
                            next_chunk // KC, next_chunk % KC)
                        next_chunk += 1

                # norm stages of the previous round fire at these k-chunks
                NORM_AT = {2: 0, 3: 1, 5: 2, 6: 3, 10: 4, 11: 5, 12: 6, 13: 7, 14: 8}
                norm_pending = []   # stage callbacks still to emit
                proj_queue = []
                warm_mms(12)   # bridge the phase-A pool-drain PE gap
                for ridx in range(NR):
                    h, qh = rounds[ridx]
                    ot_ps = ppo.tile([DH + 1, 1024], F32, tag="ot")
                    for kc in range(KC):
                        si = NORM_AT.get(kc)
                        if si is not None and si < len(norm_pending):
                            norm_pending[si]()
                        lin = ridx * KC + kc
                        if ridx >= 5 and kc in (4, 8, 12) and proj_queue:
                            emit_proj_tile(proj_queue.pop(0))
                        ensure_chunks(lin + LOOK)
                        pt_t = pt_map.pop(lin)
                        for j in range(2):
                            _mm(nc, ot_ps[:, j * 512:(j + 1) * 512],
                                vaug_sb[:, kc, h],
                                pt_t[:, j * 512:(j + 1) * 512],
                                kc == 0, kc == KC - 1)
                        if ridx == 0 and kc % 2 == 1:
                            # round 0 is mask-DMA paced; keep HAM fed
                            warm_mms(3)
                    norm_pending = norm_stages((h, qh, ot_ps))
                    if qh == 0 and h == HG - 1:
                        # qh0's ot_pair completes once the (h3,qh0) norm fires
                        # inside round (h0,qh1); queue its projection
                        proj_queue.extend(range(8))
                for i, s in enumerate(norm_pending):
                    s()
                    if i in (1, 3):
                        warm_mms(6)   # keep HAM warm through the DMA latency
                for tt in proj_queue:
                    emit_proj_tile(tt, tail=True)
                for tt in range(8, 16):
                    emit_proj_tile(tt, tail=True)

    if split_waits:
        _split_excess_waits(nc)
    return nc


_program_cache = None


def _get_program():
    global _program_cache
    if _program_cache is None:
        _program_cache = build_program()
    return _program_cache


def kernel(query, key, value, mask, Wq, Wk, Wv, Wp, bp):
    query = np.asarray(query, np.float32)
    key = np.asarray(key, np.float32)
    value = np.asarray(value, np.float32)
    mask = np.asarray(mask)
    Wq = np.asarray(Wq, np.float32)
    Wk = np.asarray(Wk, np.float32)
    Wv = np.asarray(Wv, np.float32)
    Wp = np.asarray(Wp, np.float32)
    bp = np.asarray(bp, np.float32)

    wq_scaled = Wq * np.float32(C) ** -0.5   # fold the score scale into Wq

    in_maps = []
    for c in range(8):
        b, g = c // GROUPS, c % GROUPS
        cols = slice(g * COLS, (g + 1) * COLS)
        in_maps.append({
            "xqT": np.ascontiguousarray(query[b].T).astype(ml_dtypes.float8_e4m3),
            "xkT": np.ascontiguousarray(key[b].T).astype(ml_dtypes.float8_e4m3),
            "xvT": np.ascontiguousarray(value[b].T).astype(ml_dtypes.bfloat16),
            "maskT": np.ascontiguousarray(mask[b].T).astype(ml_dtypes.bfloat16),
            "wq": np.ascontiguousarray(wq_scaled[:, cols] * 256.0).astype(ml_dtypes.float8_e4m3),
            "wk": np.ascontiguousarray(Wk[:, cols] * 256.0).astype(ml_dtypes.float8_e4m3),
            "wv": np.ascontiguousarray(Wv[:, cols]).astype(ml_dtypes.bfloat16),
            "wp": np.ascontiguousarray(Wp[cols, :]).astype(ml_dtypes.bfloat16),
            "ones": np.ones((1, DH), np.float32),
        })

    nc = _get_program()
    res = run_bass_kernel_spmd(nc, in_maps, list(range(8)))
    globals()["_last_result"] = res

    out = np.empty((B, T, C), np.float32)
    for b in range(B):
        acc = res.results[b * GROUPS]["y"].astype(np.float32)
        for g in range(1, GROUPS):
            acc = acc + res.results[b * GROUPS + g]["y"]
        out[b] = acc + bp
    return out
